# revision 19
# baseline (speedup 1.0000x reference)
"""GAT (3-layer GATConv + BatchNorm + ELU) on Trainium2, 8 NeuronCores.

Architecture (v2): aggregate-then-transform. GATConv is linear in the
source features: sum_e alpha_e (x[src_e] @ W) = (sum_e alpha_e x[src_e]) @ W.
So the edge phase gathers raw x rows (256/512 B) instead of h rows (1 KB),
aggregates per-head alpha-weighted x into z via one-hot matmuls on TensorE,
and applies W once per 128-dst block. This removes the replicated dense
phase and all h-row DRAM traffic.

Sharding: dst-slot partitioning. Host bin-packs nodes into 392 blocks of
<=128 nodes with balanced in-degree sums; blocks are dealt to 8 cores.
All indexing on device uses slot ids (node -> slot permutation applied on
the host, inverted on output).

Edge phase per 5-block group: three dma_gather instructions (x rows by
src-slot, split in two overlapping int16 sections; ad rows by block-local
dst-slot). Per 128-edge chunk: ex = exp(leakyrelu(as_src + ad_dst)) with
as carried in the gathered row (f32 packed in the bf16 row); one-hot and
per-head scaled one-hots built with batched broadcast-AP DVE ops; TensorE
accumulates zT[KIN, (h,dst)] and the softmax denominator d[dst, h] in PSUM.
Per block: u_h = z_h @ W_h, out = sum_h u_h / (H * d_h).

Between layers: BatchNorm (global stats via tiny AllReduce) + ELU + an
alpha matmul produce the next layer's packed xa rows ([x | as] bf16+f32)
and ad rows; xa is AllGathered, ad stays core-local (dst-sharded).
"""

import sys

sys.path.insert(0, "/opt/trn_rl_repo")

import numpy as np
from contextlib import ExitStack

import concourse.bass as bass
import concourse.bacc as bacc
import concourse.mybir as mybir
import concourse.tile as tile
from concourse.bass_utils import run_bass_kernel_spmd

AF = mybir.ActivationFunctionType
ALU = mybir.AluOpType
DT = mybir.dt
BF16 = np.dtype(DT.np(DT.bfloat16))

NEG_SLOPE = 0.2
BN_EPS = 1e-5


class Cfg:
    def __init__(self):
        self.N = 50000
        self.E = 400000
        self.F_IN = 64
        self.HID = 128
        self.OUT = 64
        self.HEADS = 4
        self.P = 8
        self.NBLK = 49                 # blocks per core
        self.NPCS = self.NBLK * 128    # slots per core (6272)
        self.NSLOT = self.P * self.NPCS  # 50176
        self.M0 = 5                    # sec0 chunks per block
        self.M1 = 5                    # sec1 chunks per block
        self.MB = self.M0 + self.M1    # chunks per block
        self.NCH = self.NBLK * self.MB  # 490
        self.HI_BASE = 17408           # sec1 row base (overlapping sections)
        self.LO_MAX = 32768
        self.G = 5                     # blocks per gather group


# ---------------------------------------------------------------- host prep


def _assign_slots(dst, cfg):
    """Bin-pack nodes into P*NBLK blocks (<=128 nodes, balanced in-degree).

    Returns slot_of_node [N] int64."""
    import heapq
    N, P, NBLK = cfg.N, cfg.P, cfg.NBLK
    nbins = P * NBLK
    deg = np.bincount(dst, minlength=N)
    order = np.argsort(-deg, kind="stable")
    heap = [(0, b) for b in range(nbins)]
    heapq.heapify(heap)
    bin_nodes = [[] for _ in range(nbins)]
    bin_load = np.zeros(nbins, np.int64)
    for n in order:
        while True:
            load, b = heapq.heappop(heap)
            if len(bin_nodes[b]) < 128:
                break
        bin_nodes[b].append(n)
        bin_load[b] = load + deg[n]
        if len(bin_nodes[b]) < 128:
            heapq.heappush(heap, (bin_load[b], b))
    # deal bins to (core, blk) sorted by load so same blk-index has
    # similar load on every core
    border = np.argsort(-bin_load, kind="stable")
    slot_of_node = np.full(N, -1, np.int64)
    for i, b in enumerate(border):
        core, blk = i % P, i // P
        base = core * cfg.NPCS + blk * 128
        for j, n in enumerate(bin_nodes[b]):
            slot_of_node[n] = base + j
    assert (slot_of_node >= 0).all()
    return slot_of_node


def _edge_schedule(src_slot, dst_slot, cfg):
    """Build per-core chunk schedules.

    Returns per-core lists: idx0/idx1 (int16 wrapped), idxAD (int16
    wrapped), mdlT [128, NCH] f32 (dl per edge, -1 pads)."""
    P, NBLK, M0, M1, MB = cfg.P, cfg.NBLK, cfg.M0, cfg.M1, cfg.MB
    NPCS, HI_BASE, LO_MAX = cfg.NPCS, cfg.HI_BASE, cfg.LO_MAX

    order = np.argsort(dst_slot, kind="stable")
    ds = dst_slot[order]
    ss = src_slot[order]

    idx0 = np.zeros((P, NBLK * M0 * 128), np.int16)
    idx1 = np.zeros((P, NBLK * M1 * 128), np.int16)
    idxAD = np.zeros((P, NBLK * MB * 128), np.int16)
    mdl = np.full((P, NBLK * MB * 128), -1.0, np.float32)

    blk_starts = np.arange(P * NBLK) * 128
    bounds = np.searchsorted(ds, blk_starts)
    bounds = np.append(bounds, len(ds))
    for k in range(P):
        for b in range(NBLK):
            gb = k * NBLK + b
            i0, i1 = bounds[gb], bounds[gb + 1]
            esrc = ss[i0:i1]
            edl = ds[i0:i1] - (k * NPCS + b * 128)
            lo = esrc < HI_BASE
            hi = esrc >= LO_MAX
            flex = ~lo & ~hi
            n_lo, n_hi, n_flex = lo.sum(), hi.sum(), flex.sum()
            cap0 = M0 * 128
            cap1 = M1 * 128
            f0 = min(int(n_flex), cap0 - int(n_lo))
            f0 = max(f0, int(n_flex) + int(n_hi) - cap1)
            assert 0 <= f0 <= n_flex, (k, b, n_lo, n_hi, n_flex)
            fidx = np.where(flex)[0]
            sel0 = np.concatenate([np.where(lo)[0], fidx[:f0]])
            sel1 = np.concatenate([np.where(hi)[0], fidx[f0:]])
            assert len(sel0) <= cap0 and len(sel1) <= cap1, (k, b)
            # sec0 chunks occupy j=0..M0-1, sec1 j=M0..MB-1
            s0 = idx0[k, b * cap0:(b + 1) * cap0]
            s0[: len(sel0)] = esrc[sel0]
            s1 = idx1[k, b * cap1:(b + 1) * cap1]
            s1[: len(sel1)] = esrc[sel1] - HI_BASE
            dl_blk = mdl[k, b * MB * 128:(b + 1) * MB * 128]
            dl_blk[: len(sel0)] = edl[sel0]
            dl_blk[M0 * 128: M0 * 128 + len(sel1)] = edl[sel1]
            adx = idxAD[k, b * MB * 128:(b + 1) * MB * 128]
            adx[: len(sel0)] = b * 128 + edl[sel0]
            adx[M0 * 128: M0 * 128 + len(sel1)] = b * 128 + edl[sel1]
    return idx0, idx1, idxAD, mdl


def _wrap16(vals):
    """[n] -> [128, n//16] int16 with idx i at [i%16, i//16], tiled x8."""
    n = vals.shape[-1]
    assert n % 16 == 0
    w = vals.reshape(n // 16, 16).T.astype(np.int16)
    return np.tile(w, (8, 1))


def _prep_inputs(inputs, cfg):
    N, P = cfg.N, cfg.P
    x = np.asarray(inputs["x"], np.float64)
    ei = np.asarray(inputs["edge_index"], np.int64)
    loop = np.arange(N, dtype=np.int64)
    src = np.concatenate([ei[0], loop])
    dst = np.concatenate([ei[1], loop])

    slot = _assign_slots(dst, cfg)
    src_slot = slot[src]
    dst_slot = slot[dst]
    idx0, idx1, idxAD, mdl = _edge_schedule(src_slot, dst_slot, cfg)

    W1 = np.asarray(inputs["W1"], np.float64)
    W2 = np.asarray(inputs["W2"], np.float64)
    W3 = np.asarray(inputs["W3"], np.float64)

    def fold(W, a_s, a_d, heads, ch):
        K = W.shape[0]
        Wr = W.reshape(K, heads, ch)
        was = np.stack([Wr[:, h] @ np.asarray(a_s, np.float64)[h]
                        for h in range(heads)], axis=1)
        wad = np.stack([Wr[:, h] @ np.asarray(a_d, np.float64)[h]
                        for h in range(heads)], axis=1)
        return was, wad           # [K, heads]

    was1, wad1 = fold(W1, inputs["as1"], inputs["ad1"], 4, 128)
    was2, wad2 = fold(W2, inputs["as2"], inputs["ad2"], 4, 128)
    was3, wad3 = fold(W3, inputs["as3"], inputs["ad3"], 1, 64)

    # L1 xa rows: [x(64) bf16 | as1(4) f32 | pad] = 128 bf16 cols
    as1 = (x @ was1).astype(np.float32)             # [N, 4]
    ad1 = (x @ wad1).astype(np.float32)
    xa0 = np.zeros((cfg.NSLOT, 128), BF16)
    xa0[slot, 0:64] = x.astype(np.float32).astype(BF16)
    xa0[slot, 64:72] = as1.view(BF16)

    adT0 = np.zeros((P, cfg.NPCS, 4), BF16)
    loc = slot % cfg.NPCS
    core = slot // cfg.NPCS
    ad1_b = ad1.astype(BF16)
    for k in range(P):
        m = core == k
        adT0[k][loc[m]] = ad1_b[m]

    common = {
        "xa0": xa0,
        "W1e": W1.astype(np.float32).astype(BF16),
        "W2e": W2.astype(np.float32).astype(BF16),
        "W3e": W3.astype(np.float32).astype(BF16),
        "Wsd2": np.concatenate([was2, wad2], 1).astype(np.float32).astype(BF16),
        "Wsd3": np.concatenate([was3, wad3], 1).astype(np.float32).astype(BF16),
        "iotab": np.tile(np.arange(128, dtype=np.float32), (128, 1)).astype(BF16),
        "ident": np.eye(128, dtype=np.float32),
        "ones_c": np.ones((128, 1), np.float32),
        "ones_r": np.ones((1, 128), np.float32),
        "gbe": np.stack([np.asarray(inputs["g1"], np.float32),
                         np.asarray(inputs["be1"], np.float32),
                         np.asarray(inputs["g2"], np.float32),
                         np.asarray(inputs["be2"], np.float32)], axis=1),
        "b3r": np.tile(np.asarray(inputs["b3"], np.float32), (128, 1)),
    }
    common["iotac"] = np.arange(128, dtype=np.float32).reshape(128, 1)
    common["ones_rb"] = np.ones((1, 128), np.float32).astype(BF16)
    in_maps = []
    for k in range(P):
        m = dict(common)
        m["adT0"] = adT0[k]
        m["i0"] = _wrap16(idx0[k])
        m["i1"] = _wrap16(idx1[k])
        m["mdlT"] = np.ascontiguousarray(
            mdl[k].reshape(cfg.NCH, 128).T).astype(BF16)
        m["mdlR"] = mdl[k][None, :].astype(BF16)
        in_maps.append(m)
    return in_maps, slot


# ---------------------------------------------------------------- fixups


def _fixup_waits(nc, max_dma=1, max_other=1):
    cnt = 0
    for bb in nc.main_func.blocks:
        new = []
        for inst in bb.instructions:
            si = getattr(inst, "sync_info", None)
            tn = type(inst).__name__
            lim = max_dma if ("DMA" in tn or "Dma" in tn) else max_other
            if ("Branch" not in tn and si is not None and si.on_wait
                    and len(si.on_wait) > lim):
                extra = list(si.on_wait[:-lim])
                keep = list(si.on_wait[-lim:])
                for w in extra:
                    nop = mybir.InstNoOp(
                        name=f"I-fw-{cnt}",
                        sync_info=mybir.SyncInfo(on_wait=[w], on_update=[]),
                        bass_nofuse=True,
                        engine=inst.engine,
                    )
                    cnt += 1
                    new.append(nop)
                inst.sync_info = mybir.SyncInfo(
                    on_wait=keep, on_update=list(si.on_update))
            new.append(inst)
        bb.instructions[:] = new
    return cnt


# ---------------------------------------------------------------- device


def build_nc(cfg, num_cores=None):
    P, NBLK, NSLOT, NPCS = cfg.P, cfg.NBLK, cfg.NSLOT, cfg.NPCS
    M0, M1, MB, NCH, G = cfg.M0, cfg.M1, cfg.MB, cfg.NCH, cfg.G
    HI_BASE = cfg.HI_BASE
    HID, OUT = cfg.HID, cfg.OUT
    f32, bf16, i16 = DT.float32, DT.bfloat16, DT.int16

    nc = bacc.Bacc("TRN2", num_devices=(num_cores or P))

    # parameters
    xa0 = nc.declare_dram_parameter("xa0", [NSLOT, 128], bf16, isOutput=False)
    adT0 = nc.declare_dram_parameter("adT0", [NPCS, 4], bf16, isOutput=False)
    i0p = nc.declare_dram_parameter("i0", [128, NBLK * M0 * 8], i16, isOutput=False)
    i1p = nc.declare_dram_parameter("i1", [128, NBLK * M1 * 8], i16, isOutput=False)
    mdlTp = nc.declare_dram_parameter("mdlT", [128, NCH], bf16, isOutput=False)
    mdlRp = nc.declare_dram_parameter("mdlR", [1, NBLK * MB * 128], bf16,
                                      isOutput=False)
    iotacp = nc.declare_dram_parameter("iotac", [128, 1], f32, isOutput=False)
    ones_rb = nc.declare_dram_parameter("ones_rb", [1, 128], bf16, isOutput=False)
    W1e = nc.declare_dram_parameter("W1e", [64, 512], bf16, isOutput=False)
    W2e = nc.declare_dram_parameter("W2e", [128, 512], bf16, isOutput=False)
    W3e = nc.declare_dram_parameter("W3e", [128, 64], bf16, isOutput=False)
    Wsd2 = nc.declare_dram_parameter("Wsd2", [128, 8], bf16, isOutput=False)
    Wsd3 = nc.declare_dram_parameter("Wsd3", [128, 2], bf16, isOutput=False)
    iotab = nc.declare_dram_parameter("iotab", [128, 128], bf16, isOutput=False)
    ident = nc.declare_dram_parameter("ident", [128, 128], f32, isOutput=False)
    ones_c = nc.declare_dram_parameter("ones_c", [128, 1], f32, isOutput=False)
    ones_r = nc.declare_dram_parameter("ones_r", [1, 128], f32, isOutput=False)
    gbe = nc.declare_dram_parameter("gbe", [128, 4], f32, isOutput=False)
    b3r = nc.declare_dram_parameter("b3r", [128, OUT], f32, isOutput=False)
    out3 = nc.declare_dram_parameter("out3", [NPCS, OUT], f32, isOutput=True)

    # internal DRAM
    xa1_loc = nc.dram_tensor("xa1_loc", [NPCS, 256], bf16)
    xa2_loc = nc.dram_tensor("xa2_loc", [NPCS, 256], bf16)
    xa1_all = nc.dram_tensor("xa1_all", [NSLOT, 256], bf16, addr_space="Shared")
    xa2_all = nc.dram_tensor("xa2_all", [NSLOT, 256], bf16, addr_space="Shared")
    adT1 = nc.dram_tensor("adT1", [NPCS, 4], bf16)
    adT2 = nc.dram_tensor("adT2", [NPCS, 1], bf16)
    st1_in = nc.dram_tensor("st1_in", [128, 2], f32)
    st1_out = nc.dram_tensor("st1_out", [128, 2], f32, addr_space="Shared")
    st2_in = nc.dram_tensor("st2_in", [128, 2], f32)
    st2_out = nc.dram_tensor("st2_out", [128, 2], f32, addr_space="Shared")

    groups = [list(range(P))]

    with tile.TileContext(nc, num_cores=(num_cores or P)) as tc:
        with ExitStack() as top:
            cpool = top.enter_context(tc.tile_pool(name="consts", bufs=1))
            opool = top.enter_context(tc.tile_pool(name="oreg", bufs=1))

            def cload(name, shape, dtype, par):
                t = cpool.tile(shape, dtype, tag=name)
                nc.sync.dma_start(out=t[:], in_=par[:])
                return t

            iota_s = cload("iota", [128, 128], bf16, iotab)
            ident_s = cload("ident", [128, 128], f32, ident)
            ones_s = cload("ones", [128, 1], f32, ones_c)
            onesr_s = cload("onesr", [1, 128], f32, ones_r)
            gbe_s = cload("gbe", [128, 4], f32, gbe)
            b3r_s = cload("b3r", [128, OUT], f32, b3r)
            W1_s = cload("w1", [64, 512], bf16, W1e)
            W2_s = cload("w2", [128, 512], bf16, W2e)
            W3_s = cload("w3", [128, 64], bf16, W3e)
            Wsd2_s = cload("wsd2", [128, 8], bf16, Wsd2)
            Wsd3_s = cload("wsd3", [128, 2], bf16, Wsd3)
            i0_s = cload("i0", [128, NBLK * M0 * 8], i16, i0p)
            i1_s = cload("i1", [128, NBLK * M1 * 8], i16, i1p)
            mdlT_s = cload("mdlT", [128, NCH], bf16, mdlTp)
            iotac_s = cload("iotac", [128, 1], f32, iotacp)
            onesrb_s = cload("onesrb", [1, 128], bf16, ones_rb)

            o_reg = opool.tile([128, NBLK * 128], f32, tag="oreg")

            # ---------------- edge phase --------------------------------
            def edge(layer, xa_src, adT_src, KIN, H, C, W_s, last=False):
                WX = xa_src.shape[1]          # row width in bf16 cols
                with ExitStack() as st:
                    gp = st.enter_context(tc.tile_pool(name=f"g{layer}", bufs=4))
                    bp = st.enter_context(tc.tile_pool(name=f"b{layer}", bufs=3))
                    pz = st.enter_context(
                        tc.tile_pool(name=f"pz{layer}", bufs=2, space="PSUM"))
                    pu = st.enter_context(
                        tc.tile_pool(name=f"pu{layer}", bufs=1, space="PSUM"))
                    pd = st.enter_context(
                        tc.tile_pool(name=f"pd{layer}", bufs=1, space="PSUM"))
                    pa = st.enter_context(
                        tc.tile_pool(name=f"pa{layer}", bufs=2, space="PSUM"))
                    for blk in range(NBLK):
                        # per-block gathers: <=1024 descriptors per
                        # instruction (16KB SWDGE carveout / 16B desc)
                        XA0 = gp.tile([128, M0 * WX], bf16, tag="XA0")
                        XA1 = gp.tile([128, M1 * WX], bf16, tag="XA1")
                        n0 = M0 * 128
                        nc.gpsimd.dma_gather(
                            XA0[:].rearrange("p (c e) -> p c e", e=WX),
                            xa_src[:, :],
                            i0_s[:, blk * M0 * 8:(blk + 1) * M0 * 8],
                            n0, n0, WX)
                        n1 = M1 * 128
                        nc.gpsimd.dma_gather(
                            XA1[:].rearrange("p (c e) -> p c e", e=WX),
                            xa_src[HI_BASE:, :],
                            i1_s[:, blk * M1 * 8:(blk + 1) * M1 * 8],
                            n1, n1, WX)
                        # ad[dst] for this block's 128 dsts: direct load +
                        # spread to edges via transposed one-hot matmuls
                        ad_blk = gp.tile([128, H], bf16, tag="adb")
                        nc.sync.dma_start(
                            out=ad_blk[:],
                            in_=adT_src[blk * 128:(blk + 1) * 128, 0:H])
                        mdr = gp.tile([1, MB * 128], bf16, tag="mdr")
                        nc.sync.dma_start(
                            out=mdr[:],
                            in_=mdlRp[:, blk * MB * 128:(blk + 1) * MB * 128])
                        ohT = bp.tile([128, MB * 128], bf16, tag="ohT")
                        for s0 in range(0, MB * 128, 512):
                            sw = min(512, MB * 128 - s0)
                            dlB = pd.tile([128, 512], f32, space="PSUM",
                                          tag="dlB")
                            nc.tensor.matmul(
                                out=dlB[:, :sw], lhsT=onesrb_s[:],
                                rhs=mdr[:, s0:s0 + sw], start=True, stop=True)
                            nc.vector.tensor_scalar(
                                out=ohT[:, s0:s0 + sw], in0=dlB[:, :sw],
                                scalar1=iotac_s[:], scalar2=None,
                                op0=ALU.is_equal)
                        if True:
                            # --- ex = exp(leakyrelu(as + ad)) for 10 chunks
                            v = bp.tile([128, MB * H], f32, tag="v")
                            for j in range(MB):
                                if j < M0:
                                    xt, cb = XA0, j * WX
                                else:
                                    xt, cb = XA1, (j - M0) * WX
                                as_v = xt[:, cb + KIN: cb + KIN + 2 * H] \
                                    .bitcast(f32)
                                ad_ps = pa.tile([128, H], f32, space="PSUM",
                                                tag="adps")
                                nc.tensor.matmul(
                                    out=ad_ps[:],
                                    lhsT=ohT[:, j * 128:(j + 1) * 128],
                                    rhs=ad_blk[:], start=True, stop=True)
                                nc.vector.tensor_add(
                                    out=v[:, j * H:(j + 1) * H],
                                    in0=as_v, in1=ad_ps[:])
                            vs = bp.tile([128, MB * H], f32, tag="vs")
                            nc.vector.tensor_scalar_mul(
                                out=vs[:], in0=v[:], scalar1=NEG_SLOPE)
                            lr = bp.tile([128, MB * H], f32, tag="lr")
                            nc.vector.tensor_max(out=lr[:], in0=v[:], in1=vs[:])
                            exb = bp.tile([128, MB * H], bf16, tag="exb")
                            nc.scalar.activation(out=exb[:], in_=lr[:],
                                                 func=AF.Exp)
                            # --- one-hot & scaled one-hots
                            oh = bp.tile([128, MB * 128], bf16, tag="oh")
                            nc.vector.tensor_tensor(
                                out=oh[:].rearrange("p (c j) -> p c j", c=MB),
                                in0=mdlT_s[:, blk * MB:(blk + 1) * MB]
                                    .unsqueeze(2).broadcast_to([128, MB, 128]),
                                in1=iota_s[:].unsqueeze(1)
                                    .broadcast_to([128, MB, 128]),
                                op=ALU.is_equal)
                            ohs = bp.tile([128, MB * H * 128], bf16, tag="ohs")
                            for h in range(H):
                                o_v = ohs[:].rearrange(
                                    "p (c h j) -> p c h j", c=MB, h=H)[:, :, h, :]
                                i1v = exb[:].rearrange(
                                    "p (c h) -> p c h", c=MB)[:, :, h:h + 1] \
                                    .broadcast_to([128, MB, 128])
                                nc.vector.tensor_tensor(
                                    out=o_v,
                                    in0=oh[:].rearrange("p (c j) -> p c j", c=MB),
                                    in1=i1v, op=ALU.mult)
                            # --- accumulate zT and d over chunks
                            zT_ps = pz.tile([KIN, H * 128], f32, space="PSUM",
                                            tag="zT")
                            d_ps = pz.tile([128, H], f32, space="PSUM", tag="d")
                            for j in range(MB):
                                if j < M0:
                                    xt, cb = XA0, j * WX
                                else:
                                    xt, cb = XA1, (j - M0) * WX
                                nc.tensor.matmul(
                                    out=zT_ps[:], lhsT=xt[:, cb: cb + KIN],
                                    rhs=ohs[:, j * H * 128:(j + 1) * H * 128],
                                    start=(j == 0), stop=(j == MB - 1))
                                nc.tensor.matmul(
                                    out=d_ps[:], lhsT=oh[:, j * 128:(j + 1) * 128],
                                    rhs=exb[:, j * H:(j + 1) * H],
                                    start=(j == 0), stop=(j == MB - 1))
                            # --- finalize block
                            de = bp.tile([128, H], f32, tag="de")
                            nc.vector.tensor_scalar_add(out=de[:], in0=d_ps[:],
                                                        scalar1=1e-20)
                            dr = bp.tile([128, H], f32, tag="dr")
                            nc.vector.reciprocal(out=dr[:], in_=de[:])
                            if H > 1:
                                dq = bp.tile([128, H], f32, tag="dq")
                                nc.vector.tensor_scalar_mul(
                                    out=dq[:], in0=dr[:], scalar1=1.0 / H)
                            else:
                                dq = dr
                            zTsb = bp.tile([KIN, H * 128], bf16, tag="zTsb")
                            nc.vector.tensor_copy(out=zTsb[:], in_=zT_ps[:])
                            u_ps = pu.tile([128, H * C], f32, space="PSUM",
                                           tag="u")
                            for h in range(H):
                                nc.tensor.matmul(
                                    out=u_ps[:, h * C:(h + 1) * C],
                                    lhsT=zTsb[:, h * 128:(h + 1) * 128],
                                    rhs=W_s[:, h * C:(h + 1) * C],
                                    start=True, stop=True)
                            if last:
                                t = bp.tile([128, C], f32, tag="l3t")
                                nc.vector.tensor_scalar(
                                    out=t[:], in0=u_ps[:, 0:C],
                                    scalar1=dq[:, 0:1], scalar2=None,
                                    op0=ALU.mult)
                                o3 = bp.tile([128, C], f32, tag="l3o")
                                nc.vector.tensor_add(out=o3[:], in0=t[:],
                                                     in1=b3r_s[:])
                                nc.sync.dma_start(
                                    out=out3[blk * 128:(blk + 1) * 128, :],
                                    in_=o3[:])
                            else:
                                acc = bp.tile([128, C], f32, tag="acc")
                                t2 = bp.tile([128, C], f32, tag="t2")
                                nc.vector.tensor_scalar(
                                    out=acc[:], in0=u_ps[:, 0:C],
                                    scalar1=dq[:, 0:1], scalar2=None,
                                    op0=ALU.mult)
                                for h in range(1, H):
                                    sc = dq[:, h:h + 1]
                                    if h % 2 == 1:
                                        nc.scalar.activation(
                                            out=t2[:],
                                            in_=u_ps[:, h * C:(h + 1) * C],
                                            func=AF.Copy, scale=sc)
                                    else:
                                        nc.vector.tensor_scalar(
                                            out=t2[:],
                                            in0=u_ps[:, h * C:(h + 1) * C],
                                            scalar1=sc, scalar2=None,
                                            op0=ALU.mult)
                                    tgt = (o_reg[:, blk * 128:blk * 128 + C]
                                           if h == H - 1 else acc)
                                    nc.vector.tensor_add(out=tgt, in0=acc[:],
                                                         in1=t2[:])

            # ---------------- batchnorm + elu + alphas + exchange --------
            def bn_phase(lidx, gcol, becol, st_in, st_out, xa_loc, xa_all,
                         adT_next, Wsd_s, H2):
                with ExitStack() as st:
                    bp = st.enter_context(tc.tile_pool(name=f"bn{lidx}",
                                                       bufs=2))
                    pp = st.enter_context(
                        tc.tile_pool(name=f"bnp{lidx}", bufs=1, space="PSUM"))
                    tp = st.enter_context(
                        tc.tile_pool(name=f"bnt{lidx}", bufs=2, space="PSUM"))
                    acc = bp.tile([128, HID], f32, tag="acc")
                    nc.vector.reduce_sum(
                        out=acc[:],
                        in_=o_reg[:].rearrange("p (b c) -> p c b", c=128),
                        axis=mybir.AxisListType.X)
                    acc2 = bp.tile([128, HID], f32, tag="acc2")
                    sq = bp.tile([128, HID], f32, tag="sq")
                    for b in range(NBLK):
                        ob = o_reg[:, b * 128:b * 128 + HID]
                        nc.scalar.square(out=sq[:], in_=ob)
                        if b == 0:
                            nc.vector.tensor_copy(out=acc2[:], in_=sq[:])
                        else:
                            nc.vector.tensor_add(out=acc2[:], in0=acc2[:],
                                                 in1=sq[:])
                    sp = pp.tile([128, 2], f32, space="PSUM", tag="sp")
                    nc.tensor.matmul(out=sp[:, 0:1], lhsT=acc[:], rhs=ones_s[:],
                                     start=True, stop=True)
                    nc.tensor.matmul(out=sp[:, 1:2], lhsT=acc2[:],
                                     rhs=ones_s[:], start=True, stop=True)
                    sts = bp.tile([128, 2], f32, tag="sts")
                    nc.vector.tensor_copy(out=sts[:], in_=sp[:])
                    nc.sync.dma_start(out=st_in[:], in_=sts[:])
                    nc.gpsimd.collective_compute(
                        "AllReduce", ALU.add, replica_groups=groups,
                        ins=[st_in.ap().opt()], outs=[st_out.ap().opt()])
                    stg = bp.tile([128, 2], f32, tag="stg")
                    nc.sync.dma_start(out=stg[:], in_=st_out[:])
                    mu = bp.tile([128, 1], f32, tag="mu")
                    nc.vector.tensor_scalar_mul(out=mu[:], in0=stg[:, 0:1],
                                                scalar1=1.0 / cfg.N)
                    ms = bp.tile([128, 1], f32, tag="ms")
                    nc.vector.tensor_scalar_mul(out=ms[:], in0=stg[:, 1:2],
                                                scalar1=1.0 / cfg.N)
                    mu2 = bp.tile([128, 1], f32, tag="mu2")
                    nc.scalar.square(out=mu2[:], in_=mu[:])
                    var = bp.tile([128, 1], f32, tag="var")
                    nc.vector.tensor_sub(out=var[:], in0=ms[:], in1=mu2[:])
                    vare = bp.tile([128, 1], f32, tag="vare")
                    nc.vector.tensor_scalar_add(out=vare[:], in0=var[:],
                                                scalar1=BN_EPS)
                    sd = bp.tile([128, 1], f32, tag="sd")
                    nc.scalar.activation(out=sd[:], in_=vare[:], func=AF.Sqrt)
                    rs = bp.tile([128, 1], f32, tag="rs")
                    nc.vector.reciprocal(out=rs[:], in_=sd[:])
                    ab = bp.tile([128, 2], f32, tag="ab")
                    nc.vector.tensor_mul(out=ab[:, 0:1], in0=rs[:], in1=gcol)
                    tmp = bp.tile([128, 1], f32, tag="tmp1")
                    nc.vector.tensor_mul(out=tmp[:], in0=mu[:], in1=ab[:, 0:1])
                    nc.vector.tensor_sub(out=ab[:, 1:2], in0=becol, in1=tmp[:])
                    tA_ps = pp.tile([1, 128], f32, space="PSUM", tag="tA")
                    tB_ps = pp.tile([1, 128], f32, space="PSUM", tag="tB")
                    nc.tensor.transpose(out=tA_ps[:], in_=ab[:, 0:1],
                                        identity=ident_s[:])
                    nc.tensor.transpose(out=tB_ps[:], in_=ab[:, 1:2],
                                        identity=ident_s[:])
                    abT_a = bp.tile([1, 128], f32, tag="abTa")
                    abT_b = bp.tile([1, 128], f32, tag="abTb")
                    nc.vector.tensor_copy(out=abT_a[:], in_=tA_ps[:])
                    nc.vector.tensor_copy(out=abT_b[:], in_=tB_ps[:])
                    rep_ps = pp.tile([128, 256], f32, space="PSUM", tag="rep")
                    nc.tensor.matmul(out=rep_ps[:, 0:128], lhsT=onesr_s[:],
                                     rhs=abT_a[:], start=True, stop=True)
                    nc.tensor.matmul(out=rep_ps[:, 128:256], lhsT=onesr_s[:],
                                     rhs=abT_b[:], start=True, stop=True)
                    for b in range(NBLK):
                        ob = o_reg[:, b * 128:b * 128 + HID]
                        t = bp.tile([128, HID], f32, tag="bt")
                        nc.vector.tensor_mul(out=t[:], in0=ob,
                                             in1=rep_ps[:, 0:128])
                        t2 = bp.tile([128, HID], f32, tag="bt2")
                        nc.vector.tensor_add(out=t2[:], in0=t[:],
                                             in1=rep_ps[:, 128:256])
                        m0 = bp.tile([128, HID], f32, tag="bm0")
                        nc.vector.tensor_scalar_min(out=m0[:], in0=t2[:],
                                                    scalar1=0.0)
                        em = bp.tile([128, HID], f32, tag="bem")
                        nc.scalar.activation(out=em[:], in_=m0[:], func=AF.Exp)
                        r0 = bp.tile([128, HID], f32, tag="br0")
                        nc.vector.tensor_scalar(out=r0[:], in0=t2[:],
                                                scalar1=0.0, scalar2=-1.0,
                                                op0=ALU.max, op1=ALU.add)
                        xb = bp.tile([128, HID], f32, tag="bxb")
                        nc.vector.tensor_add(out=xb[:], in0=r0[:], in1=em[:])
                        xarow = bp.tile([128, 256], bf16, tag="xarow")
                        nc.vector.tensor_copy(out=xarow[:, 0:128], in_=xb[:])
                        tr_ps = tp.tile([128, 128], f32, space="PSUM", tag="tr")
                        nc.tensor.transpose(out=tr_ps[:], in_=xb[:],
                                            identity=ident_s[:])
                        xts = bp.tile([128, 128], bf16, tag="xts")
                        nc.vector.tensor_copy(out=xts[:], in_=tr_ps[:])
                        al_ps = tp.tile([128, 2 * H2], f32, space="PSUM",
                                        tag="al")
                        nc.tensor.matmul(out=al_ps[:], lhsT=xts[:],
                                         rhs=Wsd_s[:], start=True, stop=True)
                        nc.vector.tensor_copy(
                            out=xarow[:, 128:128 + 2 * H2].bitcast(f32),
                            in_=al_ps[:, 0:H2])
                        adrow = bp.tile([128, H2], bf16, tag="adrow")
                        nc.vector.tensor_copy(out=adrow[:],
                                              in_=al_ps[:, H2:2 * H2])
                        nc.sync.dma_start(
                            out=xa_loc[b * 128:(b + 1) * 128, :],
                            in_=xarow[:])
                        nc.sync.dma_start(
                            out=adT_next[b * 128:(b + 1) * 128, :],
                            in_=adrow[:])
                    nc.gpsimd.collective_compute(
                        "AllGather", ALU.bypass, replica_groups=groups,
                        ins=[xa_loc.ap().opt()], outs=[xa_all.ap().opt()])

            # ================= network ===================================
            edge(1, xa0, adT0, 64, 4, 128, W1_s)
            tc.strict_bb_all_engine_barrier()
            bn_phase(1, gbe_s[:, 0:1], gbe_s[:, 1:2], st1_in, st1_out,
                     xa1_loc, xa1_all, adT1, Wsd2_s, 4)
            tc.strict_bb_all_engine_barrier()
            edge(2, xa1_all, adT1, 128, 4, 128, W2_s)
            tc.strict_bb_all_engine_barrier()
            bn_phase(2, gbe_s[:, 2:3], gbe_s[:, 3:4], st2_in, st2_out,
                     xa2_loc, xa2_all, adT2, Wsd3_s, 1)
            tc.strict_bb_all_engine_barrier()
            edge(3, xa2_all, adT2, 128, 1, 64, W3_s, last=True)

    nc.compile()
    _fixup_waits(nc)
    return nc


# ---------------------------------------------------------------- entry

_CACHED = {}


def _get_program(cfg):
    if "nc" not in _CACHED:
        _CACHED["nc"] = build_nc(cfg)
    return _CACHED["nc"]


def kernel(**inputs):
    cfg = Cfg()
    in_maps, slot = _prep_inputs(inputs, cfg)
    nc = _get_program(cfg)
    res = run_bass_kernel_spmd(nc, in_maps, list(range(cfg.P)))
    full = np.concatenate(
        [res.results[k]["out3"] for k in range(cfg.P)], axis=0)
    return full[slot].astype(np.float32)


if __name__ == "__main__":
    cfg = Cfg()
    rng = np.random.default_rng(0)
    dst = rng.integers(0, cfg.N, cfg.E + cfg.N)
    slot = _assign_slots(dst, cfg)
    print("slots ok:", len(np.unique(slot)) == cfg.N)
